# revision 20
# baseline (speedup 1.0000x reference)
"""MoE expert-parallel kernel for Trainium2 (8 NeuronCores, 1 expert/core).

Reference computation per expert e:
    h   = relu(x_e @ W1_e)               [N, DFF]
    agg[d] += h[src[k]] for dst[k]==d    (segment-sum over NE edges)
    out = agg @ W2_e                     [N, D]

Key transformations:
  1. segment_sum is linear:  (S @ h) @ W2 == S @ (h @ W2),
     where S[d, s] = #edges s->d.  Applying W2 *before* the aggregation
     halves the cost of the aggregation matmul (D < DFF).
  2. S is built on the host from edge_index (dense count matrix) so the
     gather/scatter becomes a dense matmul on the tensor engine.
  3. Everything runs in bf16 (same PE rate as fp32r, half the DMA and
     SBUF footprint; counts in S are small ints, exact in bf16).
     Accumulation is fp32 in PSUM; measured rel_l2 error ~5e-3.

Device pipeline per core (expert), phases A and B fused per 512-token
slice so h never leaves SBUF:
    A(ns): hT[f, n_slice] = relu( W1[d, f].T @ xT[d, n_slice] )  (K = D)
    B(ns): m[n_slice, d]  = hT[f, n_slice].T @ W2[f, d]          (K = DFF)
    C:     out[n', d]     = ST[s, n'].T @ m[s, d]                (K = N)
All of m (8.4 MB bf16) stays resident in SBUF between B and C.  W1/W2
chunks are staged just-in-time during slice 0; ST tiles are prefetched
(double-buffered) so phase C starts without a stall.

Measured: ~906,000 ns HW exec, max over 8 cores (PE-array gap-free on
7/8 cores; the 4096 512-column matmuls alone are 882 us at 2.4 GHz, so
this sits at ~97% of the tensor-engine roofline — the residue is NEFF
boot skew, drain, and PE clock jitter).  rel_l2 vs fp32 ref ~2.1e-3.
"""

import os

import numpy as np
from ml_dtypes import bfloat16

import concourse.bass as bass
import concourse.mybir as mybir
import concourse.tile as tile
from concourse import bacc
from concourse.bass_utils import run_bass_kernel_spmd

E, N, D, DFF = 8, 4096, 1024, 2048
P = 128
NT = N // P     # 32  token tiles of 128
DC = D // P     # 8   d chunks (K for phase A)
FT = DFF // P   # 16  f chunks
DS = D // 512   # 2   d slices of 512
NS = N // 512   # 8   n slices of 512 (phase A granularity)

F32 = mybir.dt.float32
BF16 = mybir.dt.bfloat16
RELU = mybir.ActivationFunctionType.Relu

_cache = {}


def _emit(nc, tc, XH, W1H, W2H, STH, out):
    with tc.tile_pool(name="w1p", bufs=1) as w1p, \
         tc.tile_pool(name="w2p", bufs=1) as w2p, \
         tc.tile_pool(name="mp", bufs=1) as mp, \
         tc.tile_pool(name="stp", bufs=2) as stp:
        msb = [None] * NT

        # ---------- fused phases A+B, one 512-token slice at a time ----------
        with tc.tile_pool(name="xp", bufs=2) as xp, \
             tc.tile_pool(name="htp", bufs=2) as htp, \
             tc.tile_pool(name="psA", bufs=4, space="PSUM") as psA, \
             tc.tile_pool(name="psB", bufs=4, space="PSUM") as psB:
            w1sb = w1p.tile([P, FT, DC, P], BF16, name="w1sb")
            w2sb = w2p.tile([P, FT, D], BF16, name="w2sb")
            xsbs = [
                xp.tile([P, DC, 512], BF16, tag="xsb", name=f"xsb{ns}")
                for ns in range(NS)
            ]
            htsbs = [
                htp.tile([P, FT, 512], BF16, tag="htsb", name=f"htsb{ns}")
                for ns in range(NS)
            ]
            # split the first loads so the first matmuls start early: the
            # first half-group (dc 0-3) gates on just 256 KB of DMA
            nc.sync.dma_start(out=w1sb[:, 0, 0:4], in_=W1H[0, :, 0:4])
            nc.sync.dma_start(out=xsbs[0][:, 0:1], in_=XH[0, :, 0:1])
            nc.sync.dma_start(out=xsbs[0][:, 1:2], in_=XH[0, :, 1:2])
            nc.sync.dma_start(out=w1sb[:, 0, 4:DC], in_=W1H[0, :, 4:DC])
            nc.sync.dma_start(out=xsbs[0][:, 2:5], in_=XH[0, :, 2:5])
            nc.sync.dma_start(out=xsbs[0][:, 5:DC], in_=XH[0, :, 5:DC])
            for ns in range(NS):
                xsb = xsbs[ns]
                htsb = htsbs[ns]
                # A(ns): hT slice = relu(W1.T @ xT slice)
                for ft in range(FT):
                    pt = psA.tile([P, 512], F32, name="ptA")
                    for dc in range(DC):
                        nc.tensor.matmul(
                            out=pt[:],
                            lhsT=w1sb[:, ft, dc, :],
                            rhs=xsb[:, dc, :],
                            start=(dc == 0),
                            stop=(dc == DC - 1),
                        )
                    nc.scalar.activation(out=htsb[:, ft], in_=pt[:], func=RELU)
                    if ns == 0:
                        # JIT-stage the next W1 chunk, then only d-half 0
                        # of this W2 chunk: B runs d-half-major, so half 1
                        # isn't needed until ~14 us into B(0).  Keeping it
                        # out of the slice-0 burst holds the per-HBM-stack
                        # demand (2 cores, SPMD-synced) under the limit.
                        if ft + 1 < FT:
                            nc.sync.dma_start(
                                out=w1sb[:, ft + 1], in_=W1H[ft + 1]
                            )
                        nc.sync.dma_start(
                            out=w2sb[:, ft, 0:512], in_=W2H[:, ft, 0:512]
                        )
                # B(ns): m tiles = hT.T @ W2, d-half-major.  The next x
                # slice is prefetched here (not at A(ns) start) so slice-0
                # weight staging isn't queued behind a 1 MB transfer it
                # doesn't need yet.
                if ns == 0:
                    nc.sync.dma_start(
                        out=w2sb[:, :, 512:D], in_=W2H[:, :, 512:D]
                    )
                if ns + 1 < NS:
                    nc.sync.dma_start(out=xsbs[ns + 1][:], in_=XH[ns + 1])
                for nt4 in range(4):
                    nt = ns * 4 + nt4
                    msb[nt] = mp.tile([P, D], BF16, tag=f"m{nt}", name=f"msb{nt}")
                for ds in range(DS):
                    for nt4 in range(4):
                        nt = ns * 4 + nt4
                        pt = psB.tile([P, 512], F32, name="ptB")
                        for fc in range(FT):
                            nc.tensor.matmul(
                                out=pt[:],
                                lhsT=htsb[:, fc, nt4 * P : (nt4 + 1) * P],
                                rhs=w2sb[:, fc, ds * 512 : (ds + 1) * 512],
                                start=(fc == 0),
                                stop=(fc == FT - 1),
                            )
                        nc.vector.tensor_copy(
                            out=msb[nt][:, ds * 512 : (ds + 1) * 512],
                            in_=pt[:],
                        )

        # ---------- phase C: out = ST.T @ m ----------
        with tc.tile_pool(name="op", bufs=3) as op, \
             tc.tile_pool(name="psC", bufs=4, space="PSUM") as psC:
            stsbs = [
                stp.tile([P, NT, P], BF16, tag="stsb", name=f"stsb{nt}")
                for nt in range(NT)
            ]
            nc.sync.dma_start(out=stsbs[0][:], in_=STH[0])
            for nt in range(NT):
                stsb = stsbs[nt]
                if nt + 1 < NT:
                    nc.sync.dma_start(out=stsbs[nt + 1][:], in_=STH[nt + 1])
                for ds in range(DS):
                    if nt == NT - 1 and ds == DS - 1:
                        # final tile: two half-column chains in separate
                        # PSUM banks, so the first half's copy+DMA overlaps
                        # the second half's matmuls — shortens the drain
                        for q in range(2):
                            c0 = ds * 512 + q * 256
                            pt = psC.tile([P, 512], F32, name="ptC")
                            for sc in range(NT):
                                nc.tensor.matmul(
                                    out=pt[:, 0:256],
                                    lhsT=stsb[:, sc, :],
                                    rhs=msb[sc][:, c0 : c0 + 256],
                                    start=(sc == 0),
                                    stop=(sc == NT - 1),
                                )
                            osb = op.tile([P, 512], F32, name="osb")
                            nc.vector.tensor_copy(
                                out=osb[:, 0:256], in_=pt[:, 0:256]
                            )
                            nc.sync.dma_start(
                                out=out[nt * P : (nt + 1) * P, c0 : c0 + 256],
                                in_=osb[:, 0:256],
                            )
                        continue
                    pt = psC.tile([P, 512], F32, name="ptC")
                    for sc in range(NT):
                        nc.tensor.matmul(
                            out=pt[:],
                            lhsT=stsb[:, sc, :],
                            rhs=msb[sc][:, ds * 512 : (ds + 1) * 512],
                            start=(sc == 0),
                            stop=(sc == NT - 1),
                        )
                    osb = op.tile([P, 512], F32, name="osb")
                    nc.vector.tensor_copy(out=osb[:], in_=pt[:])
                    nc.sync.dma_start(
                        out=out[
                            nt * P : (nt + 1) * P, ds * 512 : (ds + 1) * 512
                        ],
                        in_=osb[:],
                    )


def _build():
    nc = bacc.Bacc()

    # XH[ns, p, dc, n'] = x[ns*512 + n', dc*128 + p]
    XH = nc.dram_tensor("XH", [NS, P, DC, 512], BF16, kind="ExternalInput")
    # W1H[ft, p, dc, f'] = W1[dc*128 + p, ft*128 + f']
    W1H = nc.dram_tensor("W1H", [FT, P, DC, P], BF16, kind="ExternalInput")
    # W2H[p, fc, d] = W2[fc*128 + p, d]
    W2H = nc.dram_tensor("W2H", [P, FT, D], BF16, kind="ExternalInput")
    # STH[nt, p, sc, n''] = S_T[sc*128 + p, nt*128 + n'']
    STH = nc.dram_tensor("STH", [NT, P, NT, P], BF16, kind="ExternalInput")
    out = nc.dram_tensor("out", [N, D], F32, kind="ExternalOutput")

    with tile.TileContext(nc) as tc:
        _emit(nc, tc, XH, W1H, W2H, STH, out)

    nc.compile()
    return nc


def kernel(x, W1, W2, edge_index):
    x = np.asarray(x, dtype=np.float32)
    W1 = np.asarray(W1, dtype=np.float32)
    W2 = np.asarray(W2, dtype=np.float32)
    edge_index = np.asarray(edge_index)

    # S_T[s, d] = #edges with src==s and dst==d  (so out = S_T.T @ m)
    src = edge_index[0].astype(np.int64)
    dst = edge_index[1].astype(np.int64)
    counts = np.bincount(src * N + dst, minlength=N * N)
    S_T = counts.reshape(N, N).astype(np.float32)
    STH = np.ascontiguousarray(
        S_T.reshape(NT, P, NT, P).transpose(2, 1, 0, 3).astype(bfloat16)
    )

    XH = np.ascontiguousarray(
        x.reshape(E, NS, 512, DC, P).transpose(0, 1, 4, 3, 2).astype(bfloat16)
    )
    W1H = np.ascontiguousarray(
        W1.reshape(E, DC, P, FT, P).transpose(0, 3, 2, 1, 4).astype(bfloat16)
    )
    W2H = np.ascontiguousarray(
        W2.reshape(E, FT, P, D).transpose(0, 2, 1, 3).astype(bfloat16)
    )

    if "nc" not in _cache:
        _cache["nc"] = _build()
    nc = _cache["nc"]

    in_maps = [
        {"XH": XH[e], "W1H": W1H[e], "W2H": W2H[e], "STH": STH}
        for e in range(E)
    ]

    trace = bool(int(os.environ.get("PROBLEM_TRACE", "0")))
    res = run_bass_kernel_spmd(nc, in_maps, core_ids=list(range(E)), trace=trace)
    _cache["last_results"] = res
    return np.stack([res.results[e]["out"] for e in range(E)]).astype(np.float32)
